# revision 21
# baseline (speedup 1.0000x reference)
"""Batched cosine-similarity matrix (retrieval_knn) on 8 TRN2 NeuronCores.

reference:  out[b, n, m] = <x[b,n,:], y[b,m,:]> / max(||x[b,n]|| * ||y[b,m]||, 1e-8)
shapes:     x, y: [8, 2048, 512] f32  ->  out: [8, 2048, 2048] f32

Sharding: data-parallel over the batch dim — batch b runs on core b.
Each core receives x[b].T and y[b].T (transposed host-side during
sharding so the contraction dim d lands on SBUF partitions; fp32 DMA
transpose doesn't exist on TRN2 and PE-transposing on device would eat
the tensor engine).

Per-core kernel:
  dots  = xT.T @ yT                     (PE, tiled 128x512, k=4x128)
  ssq_x = ones.T @ (xT*xT) via N=1 MMs  -> [128,1] per n-tile (n on partitions)
  ssq_y = ones128.T @ (yT*yT)           -> [128,512] replicated across partitions
  rx = 1/sqrt(ssq_x)  (ACT sqrt + DVE reciprocal; ACT Rsqrt is banned/inaccurate)
  out_tile = (dots * rx[n]) * ry[m]     (single fused DVE scalar_tensor_tensor)
"""

import numpy as np

import concourse.bass as bass
import concourse.bacc as bacc
import concourse.mybir as mybir
import concourse.tile as tile
from concourse import bass_utils as _bu
from concourse.bass_utils import run_bass_kernel_spmd

# NOTE: walrus --enable-ldw-opt=true was tried to dedupe the per-matmul
# weight loads; it removes few LDWs and produces all-zero output for f32r
# matmuls whose LDW got hoisted (the known standalone-LDW f32r HW bug).
# Keep the default (false).

P = 128          # partitions
D = 512          # feature dim (contraction)
N = 2048         # rows of x / y
B = 8            # batch == n_cores
KC = D // P      # 4 k-chunks
NT = N // P      # 16 n-tiles (output partition tiles)
MC = N // 512    # 4 m-chunks (output free chunks, PSUM-bank width)

F32 = mybir.dt.float32

_CACHED = {}
_VARIANT = ""  # debug switches, e.g. "--norm-f32"


def _build_nc(mm_dtype: str = "float32") -> bass.Bass:
    """Build the single-core Bass program (same program runs SPMD on 8 cores)."""
    nc = bacc.Bacc(trn_type="TRN2", target_bir_lowering=False, debug=False)

    xT = nc.dram_tensor("xT", [D, N], F32, kind="ExternalInput").ap()
    yT = nc.dram_tensor("yT", [D, N], F32, kind="ExternalInput").ap()
    out = nc.dram_tensor("out", [N, N], F32, kind="ExternalOutput").ap()

    mmdt = {"float32": F32, "float32r": mybir.dt.float32r}[mm_dtype]

    with tile.TileContext(nc) as tc:
        with (
            tc.tile_pool(name="xin", bufs=1) as xin_pool,
            tc.tile_pool(name="yin", bufs=1) as yin_pool,
            tc.tile_pool(name="sq", bufs=2) as sq_pool,
            tc.tile_pool(name="consts", bufs=1) as const_pool,
            tc.tile_pool(name="norms", bufs=1) as norm_pool,
            tc.tile_pool(name="ostage", bufs=6) as out_pool,
            tc.tile_pool(name="mm_ps", bufs=4, space="PSUM") as mm_ps_pool,
            tc.tile_pool(name="ry_ps", bufs=2, space="PSUM") as ry_ps_pool,
            tc.tile_pool(name="rx_ps", bufs=2, space="PSUM") as rx_ps_pool,
        ):
            sqdt = F32  # norm pipeline stays fp32 (HW-verified; cheap)
            ones = const_pool.tile([P, P], sqdt, name="ones")
            if sqdt is F32:
                nc.vector.memset(ones, 1.0)
            else:
                ones_f = const_pool.tile([P, P], F32, name="ones_f")
                nc.vector.memset(ones_f, 1.0)
                nc.scalar.copy(ones, ones_f)

            # ---- load inputs: 4 chunks of [128, 2048] each, split into
            # [128, 512] DMAs for load/compute overlap granularity.
            # float32r matmul inputs must be pre-rounded to fp32r; gpsimd
            # (SWDGE) DMAs cast+round during the load.
            xt, yt = [], []
            dma_in = nc.sync if mmdt is F32 else nc.gpsimd
            for k in range(KC):
                xk = xin_pool.tile([P, N], mmdt, name=f"xt{k}", tag=f"xt{k}")
                yk = yin_pool.tile([P, N], mmdt, name=f"yt{k}", tag=f"yt{k}")
                for c in range(MC):
                    cs = slice(c * 512, (c + 1) * 512)
                    dma_in.dma_start(out=xk[:, cs], in_=xT[k * P:(k + 1) * P, cs])
                    dma_in.dma_start(out=yk[:, cs], in_=yT[k * P:(k + 1) * P, cs])
                xt.append(xk)
                yt.append(yk)

            # ---- norms ------------------------------------------------
            # squares (ACT engine; DVE is reserved for the epilogue).
            # Output dtype matches the matmul dtype so the norm matmuls can
            # run at f32r speed (producers must round to f32r).
            xsq, ysq = [], []
            for k in range(KC):
                xs = sq_pool.tile([P, N], sqdt, name=f"xsq{k}", tag=f"xsq{k}", bufs=1)
                ys = sq_pool.tile([P, N], sqdt, name=f"ysq{k}", tag=f"ysq{k}", bufs=1)
                nc.scalar.square(xs, xt[k].bitcast(F32))
                nc.scalar.square(ys, yt[k].bitcast(F32))
                xsq.append(xs)
                ysq.append(ys)

            # ry: ones128.T @ ysq -> [128, 512] replicated column sums; sqrt
            # on ACT (Rsqrt/Reciprocal activations are banned for accuracy),
            # reciprocal on DVE.  HW-verified path.
            ry = norm_pool.tile([P, N], F32, name="ry")
            sny = norm_pool.tile([P, N], F32, name="sny")
            for c in range(MC):
                cs = slice(c * 512, (c + 1) * 512)
                n_ps = ry_ps_pool.tile([P, 512], F32, name="n_ps", tag="n_ps")
                for k in range(KC):
                    nc.tensor.matmul(
                        n_ps, lhsT=ones, rhs=ysq[k][:, cs],
                        start=(k == 0), stop=(k == KC - 1),
                    )
                nc.scalar.sqrt(sny[:, cs], n_ps)
                nc.vector.reciprocal(ry[:, cs], sny[:, cs])

            # rx: per-partition layout [128, 16] via N=1 matmuls (column
            # sums of xsq land with n on partitions).  HW-verified path.
            rx_acc = norm_pool.tile([P, NT], F32, name="rx_acc")
            for k in range(KC):
                rx_ps = rx_ps_pool.tile([P, NT], F32, name="rx_ps", tag="rx_ps")
                for t in range(NT):
                    nc.tensor.matmul(
                        rx_ps[:, t:t + 1],
                        lhsT=xsq[k][:, t * P:(t + 1) * P],
                        rhs=ones[:, 0:1],
                        start=True, stop=True,
                    )
                if k == 0:
                    nc.vector.tensor_copy(rx_acc, rx_ps)
                else:
                    nc.vector.tensor_tensor(rx_acc, rx_acc, rx_ps,
                                            mybir.AluOpType.add)
            rx_sqrt = norm_pool.tile([P, NT], F32, name="rx_sqrt")
            nc.scalar.sqrt(rx_sqrt, rx_acc)
            rx = norm_pool.tile([P, NT], F32, name="rx")
            nc.vector.reciprocal(rx, rx_sqrt)

            # ---- main matmuls + fused epilogue ------------------------
            # Loop order t -> k -> c: the 4 c-chunks reuse one stationary
            # (xt[k][:, t-tile]) so walrus ldw-opt drops 3 of 4 LDWEIGHTS.
            for t in range(NT):
                ts_ = slice(t * P, (t + 1) * P)
                pss = [
                    mm_ps_pool.tile([P, 512], F32, name=f"ps{c}", tag=f"ps{c}",
                                    bufs=1)
                    for c in range(MC)
                ]
                for k in range(KC):
                    lhs = xt[k][:, ts_]
                    for c in range(MC):
                        nc.tensor.matmul(
                            pss[c], lhsT=lhs,
                            rhs=yt[k][:, c * 512:(c + 1) * 512],
                            start=(k == 0), stop=(k == KC - 1),
                        )
                for c in range(MC):
                    cs = slice(c * 512, (c + 1) * 512)
                    ot = out_pool.tile([P, 512], F32, name="ot", tag="ot")
                    # ot = (ps * rx[:, t]) * ry[:, m-chunk]
                    nc.vector.scalar_tensor_tensor(
                        ot, in0=pss[c], scalar=rx[:, t:t + 1], in1=ry[:, cs],
                        op0=mybir.AluOpType.mult, op1=mybir.AluOpType.mult,
                    )
                    nc.sync.dma_start(out=out[ts_, cs], in_=ot)

    nc.compile()
    return nc


def _get_nc(mm_dtype: str = "float32") -> bass.Bass:
    if mm_dtype not in _CACHED:
        _CACHED[mm_dtype] = _build_nc(mm_dtype)
    return _CACHED[mm_dtype]


def _shard(x: np.ndarray, y: np.ndarray):
    """Host-side sharding: batch b -> core b, transposed to [512, 2048]."""
    x = np.asarray(x, dtype=np.float32)
    y = np.asarray(y, dtype=np.float32)
    xTs = np.ascontiguousarray(np.transpose(x, (0, 2, 1)))
    yTs = np.ascontiguousarray(np.transpose(y, (0, 2, 1)))
    return [{"xT": xTs[b], "yT": yTs[b]} for b in range(B)]


def _run(x: np.ndarray, y: np.ndarray, mm_dtype: str = "float32",
         trace: bool = False):
    """Returns (out [8, 2048, 2048] f32, BassKernelResults)."""
    nc = _get_nc(mm_dtype)
    in_maps = _shard(x, y)
    res = run_bass_kernel_spmd(nc, in_maps, core_ids=list(range(B)), trace=trace)
    out = np.stack([res.results[b]["out"] for b in range(B)])
    return out, res


def kernel(x: np.ndarray, y: np.ndarray) -> np.ndarray:
    out, _ = _run(x, y, mm_dtype="float32")
    return out
